# revision 1
# baseline (speedup 1.0000x reference)
"""LSTM warmup+autoregressive-decode kernel for 8 Trainium2 NeuronCores.

Strategy (tensor-parallel over the 4U gate dimension):
  - Each core owns a 256-feature slice of U (same slice of each gate i,f,g,o).
  - Transposed layout everywhere: features on SBUF partitions, batch on the
    free (moving) dimension -> 512-wide moving operands at fp16 full rate.
  - Warmup step: z^T = Kslice^T x_t^T + Rslice^T h^T accumulated in PSUM
    (fp32), gates on ScalarE (sigmoid/tanh with the bias folded in), c-state
    kept fp32 on VectorE, h slice written fp16.
  - h is all-gathered every step in 2 chunks of [128,512] so the second
    chunk's collective overlaps the first chunk's matmuls of the next step.
  - Decode folds the feedback path: z = h @ (rec + dense_w @ kernel) + b_dec
    (host-precomputed fold), so only one 16-k-tile matmul per decode step is
    on the critical path; pred_t = h_t @ dense_w + dense_b is computed from
    the gathered h right after each all-gather (off the critical path).
  - Weight matrices' h-input ROWS are permuted on the host to match the
    rank-concatenated all-gather layout.

kernel(**inputs) takes the full unsharded inputs and returns [B, OUT, F].
"""

import sys, time as _time

for _p in ("/opt/trn_rl_repo", "/root/.axon_site/_ro/trn_rl_repo"):
    if _p not in sys.path:
        sys.path.insert(0, _p)

import os

import numpy as np

import concourse.bass as bass
import concourse.mybir as mybir
import concourse.tile as tile
from concourse import bacc
from concourse.bass import ts
from concourse.bass_utils import run_bass_kernel_spmd

B, T, F, U = 512, 48, 2048, 2048
OUT_STEPS = 24
W = 8  # cores
USL = U // W  # 256 features of each gate per core
MSL = 4 * USL  # 1024 gate columns per core
KT = F // 128  # 16 k-tiles over the x/h feature dim
MT = MSL // 128  # 8 m-tiles per core slice
NCHUNK = 2  # h all-gather chunks per step (128 features each)
FP16 = mybir.dt.float16
FP32 = mybir.dt.float32
AF = mybir.ActivationFunctionType

# m-tile index of each gate sub-block within the slice columns
# slice cols: [i(0:256) | f(256:512) | g(512:768) | o(768:1024)]
GI, GF, GG, GO = 0, 2, 4, 6

_last_results = {"exec_time_ns": None}


def build_nc(t_warm=T, t_dec=OUT_STEPS - 1, trace_scopes=False):
    nc = bacc.Bacc("TRN2", target_bir_lowering=False, debug=False, num_devices=W)

    k_in = nc.dram_tensor("k_sl", [KT, 128, MSL], FP16, kind="ExternalInput")
    r_in = nc.dram_tensor("r_sl", [KT, 128, MSL], FP16, kind="ExternalInput")
    wd_in = nc.dram_tensor("wd_sl", [KT, 128, MSL], FP16, kind="ExternalInput")
    dw_in = nc.dram_tensor("dw_sl", [KT, 128, USL], FP16, kind="ExternalInput")
    bias_in = nc.dram_tensor("bias_sl", [MT, 128], FP32, kind="ExternalInput")
    bdec_in = nc.dram_tensor("bdec_sl", [MT, 128], FP32, kind="ExternalInput")
    db_in = nc.dram_tensor("db_sl", [USL // 128, 128], FP32, kind="ExternalInput")
    assert t_warm % W == 0
    xsh = t_warm // W
    x_in = nc.dram_tensor("x_t", [xsh, KT, 128, B], FP16, kind="ExternalInput")
    p_out = nc.dram_tensor(
        "preds", [t_dec + 1, USL // 128, 128, B], FP16, kind="ExternalOutput"
    )

    with tile.TileContext(nc) as tc:
        with (
            tc.tile_pool(name="wpool", bufs=1) as wpool,
            tc.tile_pool(name="state", bufs=1) as state,
            tc.tile_pool(name="hbufs", bufs=2) as hbufs,
            tc.tile_pool(name="xbufs", bufs=2) as xbufs,
            tc.tile_pool(name="gtmp", bufs=2) as gtmp,
            tc.tile_pool(name="outp", bufs=4) as outp,
            tc.tile_pool(name="zps", bufs=6, space="PSUM") as zps,
            tc.tile_pool(name="pps", bufs=2, space="PSUM") as pps,
            tc.tile_pool(name="agin", bufs=4, space="DRAM") as agin,
            tc.tile_pool(name="agout", bufs=4, space="DRAM") as agout,
        ):
            # --- resident weights ---
            ksl = wpool.tile([128, KT, MSL], FP16, tag="kw", bufs=1)
            rsl = wpool.tile([128, KT, MSL], FP16, tag="rsl")
            dwsl = wpool.tile([128, KT, USL], FP16, tag="dwsl")
            bias = wpool.tile([128, MT], FP32, tag="bias")
            bdec = wpool.tile([128, MT], FP32, tag="bdec")
            dbsl = wpool.tile([128, USL // 128], FP32, tag="dbsl")
            nc.sync.dma_start(ksl[:], k_in.rearrange("k p m -> p k m"))
            nc.sync.dma_start(rsl[:], r_in.rearrange("k p m -> p k m"))
            nc.sync.dma_start(dwsl[:], dw_in.rearrange("k p m -> p k m"))
            nc.sync.dma_start(bias[:], bias_in.rearrange("m p -> p m"))
            nc.sync.dma_start(bdec[:], bdec_in.rearrange("m p -> p m"))
            nc.sync.dma_start(dbsl[:], db_in.rearrange("m p -> p m"))

            # --- x all-gather: each core ships t_warm/W steps; gather on-device.
            # One AG per within-shard step s so step 0 only waits for AG_0.
            # gathered layout: xg[s][r] = global step r*xsh + s.
            xg = []
            for s_i in range(xsh):
                xb = agin.tile([KT * 128, B], FP16, tag="xagin", name=f"xb{s_i}")
                nc.sync.dma_start(
                    xb[:], x_in[s_i].rearrange("k p n -> (k p) n")
                )
                xo = agout.tile(
                    [W * KT * 128, B],
                    FP16,
                    addr_space="Shared",
                    name=f"xo{s_i}",
                    tag="xo",
                    bufs=xsh,
                )
                nc.gpsimd.collective_compute(
                    "AllGather",
                    mybir.AluOpType.bypass,
                    replica_groups=[list(range(W))],
                    ins=[xb[:].opt()],
                    outs=[xo[:].opt()],
                )
                xg.append(xo.rearrange("(r k p) n -> r k p n", r=W, p=128))

            # --- persistent state: c (fp32), 2 chunks of 128 features ---
            c_st = [state.tile([128, B], FP32, tag=f"c{j}", name=f"c_st{j}") for j in range(NCHUNK)]
            for cs in c_st:
                nc.vector.memset(cs[:], 0.0)

            def gather_h(h_tiles, hbuf_next):
                """AllGather the NCHUNK h-slice tiles into hbuf_next[:, :, :]."""
                for c in range(NCHUNK):
                    bi = agin.tile([128, B], FP16, tag="agin")
                    go = agout.tile([W * 128, B], FP16, tag="agout")
                    nc.sync.dma_start(bi[:], h_tiles[c][:])
                    if os.environ.get("SKIP_AG"):
                        nc.sync.dma_start(go[0:128, :], bi[:])
                    else:
                        nc.gpsimd.collective_compute(
                            "AllGather",
                            mybir.AluOpType.bypass,
                            replica_groups=[list(range(W))],
                            ins=[bi[:].opt()],
                            outs=[go[:].opt()],
                        )
                    nc.sync.dma_start(
                        hbuf_next[:, c * W : (c + 1) * W, :],
                        go.rearrange("(r p) n -> p r n", p=128),
                    )

            def lstm_step(z_mm, step_bias):
                """Emit gates+state update. z_mm(m) emits matmuls into a PSUM
                tile for m-tile m and returns it. Returns h tiles (fp16)."""
                h_tiles = []
                for c in range(NCHUNK):
                    zi = z_mm(GI + c)
                    zf = z_mm(GF + c)
                    zg = z_mm(GG + c)
                    zo = z_mm(GO + c)
                    si = gtmp.tile([128, B], FP16, tag="si")
                    sf = gtmp.tile([128, B], FP16, tag="sf")
                    tg = gtmp.tile([128, B], FP16, tag="tg")
                    so = gtmp.tile([128, B], FP16, tag="so")
                    nc.scalar.activation(
                        si[:], zi[:], AF.Sigmoid, bias=step_bias[:, GI + c : GI + c + 1]
                    )
                    nc.scalar.activation(
                        sf[:], zf[:], AF.Sigmoid, bias=step_bias[:, GF + c : GF + c + 1]
                    )
                    nc.scalar.activation(
                        tg[:], zg[:], AF.Tanh, bias=step_bias[:, GG + c : GG + c + 1]
                    )
                    nc.scalar.activation(
                        so[:], zo[:], AF.Sigmoid, bias=step_bias[:, GO + c : GO + c + 1]
                    )
                    t1 = gtmp.tile([128, B], FP32, tag="t1")
                    t2 = gtmp.tile([128, B], FP32, tag="t2")
                    nc.vector.tensor_tensor(
                        t1[:], sf[:], c_st[c][:], mybir.AluOpType.mult
                    )
                    nc.vector.tensor_tensor(t2[:], si[:], tg[:], mybir.AluOpType.mult)
                    nc.vector.tensor_tensor(
                        c_st[c][:], t1[:], t2[:], mybir.AluOpType.add
                    )
                    tc_ = gtmp.tile([128, B], FP16, tag="tc")
                    nc.scalar.activation(tc_[:], c_st[c][:], AF.Tanh)
                    h_j = gtmp.tile([128, B], FP16, tag=f"h{c}", name=f"h_j{c}")
                    nc.vector.tensor_tensor(h_j[:], so[:], tc_[:], mybir.AluOpType.mult)
                    h_tiles.append(h_j)
                return h_tiles

            def emit_pred(hbuf, t_idx):
                """pred_t slice = dense_w_sl^T @ h_full (+ dense_b), to DRAM."""
                for m2 in range(USL // 128):
                    pp = pps.tile([128, B], FP32, tag="pp")
                    for k in range(KT):
                        nc.tensor.matmul(
                            pp[:],
                            dwsl[:, k, ts(m2, 128)],
                            hbuf[:, k, :],
                            start=(k == 0),
                            stop=(k == KT - 1),
                        )
                    po = outp.tile([128, B], FP16, tag="po")
                    nc.scalar.activation(
                        po[:], pp[:], AF.Identity, bias=dbsl[:, m2 : m2 + 1]
                    )
                    nc.sync.dma_start(p_out[t_idx, m2], po[:])

            # ---------------- warmup ----------------
            hbuf = None
            for t in range(t_warm):
                xt = xbufs.tile([128, KT, B], FP16, tag="xt")
                nc.sync.dma_start(xt[:], xg[t % xsh][t // xsh].rearrange("k p n -> p k n"))

                def z_mm(m, xt=xt, hbuf=hbuf, first=(t == 0)):
                    zp = zps.tile([128, B], FP32, tag="z")
                    for k in range(KT):
                        nc.tensor.matmul(
                            zp[:],
                            ksl[:, k, ts(m, 128)],
                            xt[:, k, :],
                            start=(k == 0),
                            stop=first and (k == KT - 1),
                        )
                    if not first:
                        for k in range(KT):
                            nc.tensor.matmul(
                                zp[:],
                                rsl[:, k, ts(m, 128)],
                                hbuf[:, k, :],
                                start=False,
                                stop=(k == KT - 1),
                            )
                    return zp

                h_tiles = lstm_step(z_mm, bias)
                hbuf_next = hbufs.tile([128, KT, B], FP16, tag="hbuf")
                gather_h(h_tiles, hbuf_next)
                hbuf = hbuf_next

            # decode weights reuse ksl's SBUF slot (warmup-only vs decode-only)
            wdsl = wpool.tile([128, KT, MSL], FP16, tag="kw", bufs=1, name="wdsl")
            nc.sync.dma_start(wdsl[:], wd_in.rearrange("k p m -> p k m"))

            # pred_0 from the final warmup h
            emit_pred(hbuf, 0)

            # ---------------- decode ----------------
            for t in range(t_dec):

                def z_mm(m, hbuf=hbuf):
                    zp = zps.tile([128, B], FP32, tag="z")
                    for k in range(KT):
                        nc.tensor.matmul(
                            zp[:],
                            wdsl[:, k, ts(m, 128)],
                            hbuf[:, k, :],
                            start=(k == 0),
                            stop=(k == KT - 1),
                        )
                    return zp

                h_tiles = lstm_step(z_mm, bdec)
                hbuf_next = hbufs.tile([128, KT, B], FP16, tag="hbuf")
                gather_h(h_tiles, hbuf_next)
                hbuf = hbuf_next
                emit_pred(hbuf, t + 1)

    nc.compile()
    return nc


def _row_perm():
    # gathered h row order: [chunk c][rank r][128 features]
    return np.array(
        [
            256 * r + 128 * c + j
            for c in range(NCHUNK)
            for r in range(W)
            for j in range(128)
        ],
        dtype=np.int64,
    )


def _slice_cols(k):
    return np.array(
        [g * U + USL * k + j for g in range(4) for j in range(USL)], dtype=np.int64
    )


def _prep_inputs(inputs, kernel, rec_kernel, bias, dense_w, dense_b, t_warm):
    x = np.asarray(inputs, np.float32)
    kern = np.asarray(kernel, np.float32)
    rec = np.asarray(rec_kernel, np.float32)
    bias = np.asarray(bias, np.float32)
    dw = np.asarray(dense_w, np.float32)
    db = np.asarray(dense_b, np.float32)

    perm = _row_perm()
    rec_p = rec[perm]
    wdec_p = (rec + dw @ kern)[perm]
    dw_p = dw[perm]
    bdec = bias + db @ kern

    # x^T: [t, k-tile, 128, B] fp16
    xT = (
        np.ascontiguousarray(np.transpose(x[:, :t_warm, :], (1, 2, 0)))
        .reshape(t_warm, KT, 128, B)
        .astype(np.float16)
    )
    xsh = t_warm // W
    x_shards = [np.ascontiguousarray(xT[c * xsh : (c + 1) * xsh]) for c in range(W)]

    in_maps = []
    for c in range(W):
        cols = _slice_cols(c)
        in_maps.append(
            {
                "k_sl": kern[:, cols].reshape(KT, 128, MSL).astype(np.float16),
                "r_sl": rec_p[:, cols].reshape(KT, 128, MSL).astype(np.float16),
                "wd_sl": wdec_p[:, cols].reshape(KT, 128, MSL).astype(np.float16),
                "dw_sl": dw_p[:, c * USL : (c + 1) * USL]
                .reshape(KT, 128, USL)
                .astype(np.float16),
                "bias_sl": bias[cols].reshape(MT, 128).astype(np.float32),
                "bdec_sl": bdec[cols].reshape(MT, 128).astype(np.float32),
                "db_sl": db[c * USL : (c + 1) * USL]
                .reshape(USL // 128, 128)
                .astype(np.float32),
                "x_t": x_shards[c],
            }
        )
    return in_maps


def kernel(
    inputs, kernel, rec_kernel, bias, dense_w, dense_b, t_warm=T, t_dec=OUT_STEPS - 1, trace=False
):
    in_maps = _prep_inputs(
        inputs, kernel, rec_kernel, bias, dense_w, dense_b, t_warm
    )
    nc = build_nc(t_warm=t_warm, t_dec=t_dec)
    _t0 = _time.time()
    res = run_bass_kernel_spmd(
        nc, in_maps, core_ids=list(range(W)), trace=trace
    )
    _wall_ns = int((_time.time() - _t0) * 1e9)
    # no NTFF hook under axon: fall back to wall clock of the SPMD dispatch
    # (includes one-time NEFF compile on a cold cache; see bench.py for the
    # warm-executable timing, ~127ms incl ~95ms axon dispatch overhead)
    _last_results["exec_time_ns"] = (
        res.exec_time_ns if res.exec_time_ns is not None else _wall_ns
    )
    _last_results["bass_results"] = res

    n_out = t_dec + 1
    preds = np.empty((B, n_out, F), np.float32)
    for c in range(W):
        o = res.results[c]["preds"].astype(np.float32)  # [n_out, USL//128, 128, B]
        preds[:, :, c * USL : (c + 1) * USL] = o.transpose(3, 0, 1, 2).reshape(
            B, n_out, USL
        )
    return preds

